# revision 12
# baseline (speedup 1.0000x reference)
"""Trainium2 Bass kernel for GPT-NeoX-style attention block (nn_Attention_88141318848873).

Full inputs -> head-parallel tensor-parallel across 8 NeuronCores -> full output.

v2: merged A/B software pipeline.
  - Phase A (QKV proj + partial RoPE + PE-transpose to [dim, tok]) emits per
    128-token tile; phase B blocks (scores -> exp -> PV) are emitted as soon as
    their k/q tiles are transposed, so the exp stream (ScalarE, ~160us total)
    hides under the PE-bound QKV stream instead of pacing a separate phase.
  - PV operand swap: V_aug [k, 64+ones] is the stationary operand, P^T [k, q]
    the moving one -> acc = attn^T [dims+den, q] accumulates over ki with a
    natural start=True bank-open (no zeroing matmuls) and the out-projection
    consumes acc's layout directly (no PE transposes of attn).
  - Normalization in [dims, q] layout: DVE reciprocal of the two denominator
    rows -> bf16 [2, 512] -> K=2 selector matmul (sel2^T @ recb) broadcasts
    recipA to partitions 0..63 / recipB to 64..127 in one 512-cycle matmul ->
    two mixed-partition-base DVE muls produce the bf16 aT pair tile.
  - All out-proj chunks deferred to the tail section (priority-last): the list
    scheduler uses them to fill PE slack while the final blocks' exp drains.
  - PSUM is sectioned (16KB/partition, bank-aligned by construction):
    s1 A-only double-buffered [0:9K); s2 scores 4K + acc 2x2K + bc 2K @0 with
    single-buffered A pool @10K; s3 swaps the A pool for the out-proj pool.
  - PE warmup: ~20 zero matmuls + a dummy exp (ACT table preload) at priority
    0 run during the initial DMA wait, flipping the HAM clock gate to 2.4GHz
    before the first real matmul; first xt tile DMA'd in quarters.
Host: shards/pre-transposes/casts inputs (scale 1/sqrt(hd) folded into Wq,
rope cos/sin tables partition-linear), sums the 8 bf16 partial outputs in f32.
"""
import sys

sys.path.insert(0, "/opt/trn_rl_repo")

import numpy as np
import ml_dtypes

import concourse.bass as bass
import concourse.mybir as mybir
import concourse.tile as tile
from concourse.bacc import Bacc
from concourse.bass_utils import run_bass_kernel_spmd
from concourse.masks import make_identity

B, S_FULL, H = 2, 2048, 2048
NH, HD, ROT = 32, 64, 16
THETA = 10000.0
NCORES = 8
HPC = NH // NCORES            # heads per core = 4
LDIM = HPC * HD               # local attn dims = 256
NEG = -1e30

bf16 = mybir.dt.bfloat16
f32 = mybir.dt.float32
nbf16 = ml_dtypes.bfloat16
Exp = mybir.ActivationFunctionType.Exp


# --------------------------------------------------------------------------
# Bass program (identical on every core; per-core tensors differ)
# --------------------------------------------------------------------------

def build_nc(S=S_FULL):
    assert S % 512 == 0
    T = B * S
    TT = T // 128                 # token tiles total
    TPB = S // 128                # token tiles per batch
    NQB = S // 512                # 512-wide q blocks per batch
    HC = H // 128                 # h (contraction) chunks

    nc = Bacc()
    xT_d = nc.dram_tensor("xT", [H, T], bf16, kind="ExternalInput")
    wqkv_d = nc.dram_tensor("wqkvT", [H, 768], bf16, kind="ExternalInput")
    wo_d = nc.dram_tensor("woT", [LDIM, H], bf16, kind="ExternalInput")
    cs_d = nc.dram_tensor("csd", [128, TT * 2 * ROT], bf16, kind="ExternalInput")
    mask_d = nc.dram_tensor("maskd", [128, 128], bf16, kind="ExternalInput")
    sel_d = nc.dram_tensor("sel33d", [33, 128], bf16, kind="ExternalInput")
    out_d = nc.dram_tensor("out", [T, H], bf16, kind="ExternalOutput")

    with tile.TileContext(nc) as tc:
        with tc.tile_pool(name="const", bufs=1) as cpool:
            wqkv_sb = cpool.tile([128, HC, 768], bf16)
            wqkv_r = wqkv_d.rearrange("(c p) d -> p c d", p=128)
            with tc.high_priority():
                nc.sync.dma_start(out=wqkv_sb[:, 0, :], in_=wqkv_r[:, 0, :])
            for hcd in range(1, HC):
                nc.sync.dma_start(
                    out=wqkv_sb[:, hcd, :], in_=wqkv_r[:, hcd, :])
            wo_sb = cpool.tile([128, 2, H], bf16)   # DMA deferred past startup
            cs_sb = cpool.tile([128, TT, 2 * ROT], bf16)
            nc.sync.dma_start(
                out=cs_sb, in_=cs_d.rearrange("p (t d) -> p t d", d=2 * ROT))
            mask_sb = cpool.tile([128, 128], bf16)   # 0/1 keep-mask, post-exp
            nc.sync.dma_start(out=mask_sb, in_=mask_d[:, :])
            ident = cpool.tile([128, 128], bf16)
            make_identity(nc, ident)
            zeros_sb = cpool.tile([128, 512], bf16)
            nc.vector.memset(zeros_sb, 0.0)
            # selector for the K=33 recip-broadcast matmul: row 0 selects
            # out partitions 0..63 (recipA), row 32 selects 64..127 (recipB)
            sel33_sb = cpool.tile([33, 128], bf16)
            nc.sync.dma_start(out=sel33_sb, in_=sel_d[:, :])

            qkT_sb = cpool.tile([128, 4, T], bf16)       # dims x tok (4 dtiles)
            V_sb = cpool.tile([128, TT, HPC, 66], bf16)  # tok x head x (64+one)
            nc.vector.memset(V_sb[:, :, :, 64:65], 1.0)

            # PE warmup + ACT exp-table preload during the startup DMA wait
            dumm = cpool.tile([1, 1], bf16)
            with tc.tile_pool(name="warm", bufs=1, space="PSUM") as wpool:
                wps = wpool.tile([128, 512], f32, tag="w")
                with tc.high_priority():
                    nc.scalar.activation(out=dumm, in_=zeros_sb[0:1, 0:1],
                                         func=Exp)
                    for _ in range(20):
                        nc.tensor.matmul(wps, zeros_sb[:, 0:128], zeros_sb,
                                         start=True, stop=True)

            sb_pools = [
                tc.tile_pool(name="xt", bufs=4),
                tc.tile_pool(name="qknat", bufs=3),
                tc.tile_pool(name="ropetmp", bufs=4),
                tc.tile_pool(name="ppool", bufs=6),
                tc.tile_pool(name="anpool", bufs=16),
                tc.tile_pool(name="recpool", bufs=4),
                tc.tile_pool(name="obpool", bufs=6),
            ]
            xpool, qpool, rpool, ppool, anpool, recpool, obpool = \
                [p.__enter__() for p in sb_pools]

            xT_r = xT_d.rearrange("(c p) t -> p c t", p=128)

            def emit_A_tile(gt, pspool, tppool, psbufs, copies_on_scalar,
                            tpbufs=4):
                xt = xpool.tile([128, HC, 128], bf16, tag="xt")
                if gt == 0:
                    with tc.high_priority():
                        for qtr in range(4):
                            nc.sync.dma_start(
                                out=xt[:, qtr * 4:(qtr + 1) * 4, :],
                                in_=xT_r[:, qtr * 4:(qtr + 1) * 4,
                                         0:128])
                elif gt < 4:
                    with tc.high_priority():
                        nc.sync.dma_start(
                            out=xt, in_=xT_r[:, :, gt * 128:(gt + 1) * 128])
                else:
                    nc.sync.dma_start(
                        out=xt, in_=xT_r[:, :, gt * 128:(gt + 1) * 128])
                ps = pspool.tile([128, 1024], f32, tag="ps", bufs=psbufs)
                for hc in range(HC):
                    nc.tensor.matmul(
                        ps[:, 0:512], xt[:, hc, :], wqkv_sb[:, hc, 0:512],
                        start=(hc == 0), stop=(hc == HC - 1))
                    nc.tensor.matmul(
                        ps[:, 512:768], xt[:, hc, :], wqkv_sb[:, hc, 512:768],
                        start=(hc == 0), stop=(hc == HC - 1))
                nc.vector.tensor_copy(
                    V_sb[:, gt, :, 0:64],
                    ps[:, 512:768].rearrange("p (h d) -> p h d", d=64))
                qk = qpool.tile([128, 512], bf16, tag="qk")
                nc.vector.tensor_copy(qk, ps[:, 0:512])
                # partial RoPE on dims 0..15 of each of the 8 (q/k, head) blocks
                rot = qk.rearrange("p (b d) -> p b d", d=64)[:, :, 0:ROT]
                rot_lo = qk.rearrange("p (b d) -> p b d", d=64)[:, :, 0:8]
                rot_hi = qk.rearrange("p (b d) -> p b d", d=64)[:, :, 8:16]
                cos_bc = cs_sb[:, gt, None, 0:ROT].broadcast_to([128, 8, ROT])
                sin_lo = cs_sb[:, gt, None, ROT:ROT + 8].broadcast_to([128, 8, 8])
                sin_hi = cs_sb[:, gt, None, ROT + 8:ROT + 16].broadcast_to([128, 8, 8])
                tmp = rpool.tile([128, 8, ROT], bf16, tag="t0")
                t2l = rpool.tile([128, 8, 8], bf16, tag="t1")
                t2h = rpool.tile([128, 8, 8], bf16, tag="t2")
                nc.vector.tensor_mul(tmp, rot, cos_bc)
                nc.vector.tensor_mul(t2l, rot_hi, sin_lo)
                nc.vector.tensor_mul(t2h, rot_lo, sin_hi)
                nc.vector.tensor_sub(rot_lo, tmp[:, :, 0:8], t2l)
                nc.vector.tensor_add(rot_hi, tmp[:, :, 8:16], t2h)
                # transpose the 4 dim-tiles into qkT
                for dt in range(4):
                    tp = tppool.tile([128, 128], bf16, tag="tp", bufs=tpbufs)
                    nc.tensor.transpose(
                        tp, qk[:, dt * 128:(dt + 1) * 128], ident)
                    if copies_on_scalar and dt % 2 == 1:
                        nc.scalar.copy(
                            qkT_sb[:, dt, gt * 128:(gt + 1) * 128], tp)
                    else:
                        nc.vector.tensor_copy(
                            qkT_sb[:, dt, gt * 128:(gt + 1) * 128], tp)

            aTs = {}

            def emit_B_block(b, qb, spool, TPB):
                for pr in range(2):
                    hA, hB = 2 * pr, 2 * pr + 1
                    accA = spool.tile([128, 512], f32, tag="a0", bufs=1)
                    accB = spool.tile([128, 512], f32, tag="a1", bufs=1)
                    nki = 4 * qb + 4
                    for ki in range(nki):
                        off = max(0, ki * 128 - qb * 512)
                        kcol = b * S + ki * 128
                        qcol = b * S + qb * 512
                        sAB = spool.tile([128, 2, 512], f32, tag="s", bufs=1)
                        with tc.high_priority(offset=150):
                            nc.tensor.matmul(
                                sAB[:, 0, off:512],
                                qkT_sb[0:64, 2 + pr, kcol:kcol + 128],
                                qkT_sb[0:64, pr, qcol + off:qcol + 512],
                                start=True, stop=True,
                                tile_position=(0, 0))
                            nc.tensor.matmul(
                                sAB[:, 1, off:512],
                                qkT_sb[64:128, 2 + pr, kcol:kcol + 128],
                                qkT_sb[64:128, pr, qcol + off:qcol + 512],
                                start=True, stop=True,
                                tile_position=(64, 0))
                        pAB = ppool.tile([128, 2, 512], bf16, tag="p")
                        nc.scalar.activation(
                            out=pAB[:, :, off:512],
                            in_=sAB[:, :, off:512], func=Exp)
                        if ki * 128 >= qb * 512:  # in-block diagonal
                            mask2 = mask_sb[:, None, :].broadcast_to(
                                [128, 2, 128])
                            nc.vector.tensor_mul(
                                pAB[:, :, off:off + 128],
                                pAB[:, :, off:off + 128], mask2)
                        last = (ki == nki - 1)
                        ti = b * TPB + ki
                        nc.tensor.matmul(
                            accA[0:65, off:512],
                            V_sb[:, ti, hA, 0:65], pAB[:, 0, off:512],
                            start=(ki == 0), stop=last)
                        nc.tensor.matmul(
                            accB[0:65, off:512],
                            V_sb[:, ti, hB, 0:65], pAB[:, 1, off:512],
                            start=(ki == 0), stop=last)
                    # normalize -> aT pair tile [128 dims(A|B), 512 q]
                    rec33 = recpool.tile([33, 512], f32, tag="rec")
                    nc.vector.reciprocal(rec33[0:1, :], accA[64:65, :])
                    nc.vector.reciprocal(rec33[32:33, :], accB[64:65, :])
                    recb33 = recpool.tile([33, 512], bf16, tag="recb")
                    nc.vector.memset(recb33, 0.0)
                    nc.vector.tensor_copy(recb33[0:1, :], rec33[0:1, :])
                    nc.vector.tensor_copy(recb33[32:33, :], rec33[32:33, :])
                    bc = spool.tile([128, 512], f32, tag="bc", bufs=1)
                    nc.tensor.matmul(bc, sel33_sb, recb33,
                                     start=True, stop=True)
                    # DVE cannot read two PSUM operands -> stage bc in SBUF
                    bcs = recpool.tile([128, 512], f32, tag="bcs")
                    nc.vector.tensor_copy(bcs, bc)
                    aT = anpool.tile([128, 512], bf16, tag="aT")
                    nc.vector.tensor_mul(aT[0:64, :], accA[0:64, :],
                                         bcs[0:64, :])
                    nc.vector.tensor_mul(aT[64:128, :], accB[0:64, :],
                                         bcs[64:128, :])
                    aTs[(b, qb, pr)] = aT

            # ---------------- section 1: A(b0) tiles 0..13, PSUM double-buf
            pA2 = tc.tile_pool(name="pA2", bufs=1, space="PSUM")
            pA2p = pA2.__enter__()
            for gt in range(0, 14):
                emit_A_tile(gt, pA2p, pA2p, 2, True)
            pA2.__exit__(None, None, None)

            # wo load deferred here so startup DMA bandwidth goes to xT/wqkv
            nc.sync.dma_start(
                out=wo_sb, in_=wo_d.rearrange("(c p) d -> p c d", p=128))

            # ---------------- section 2: B pools + single-buffered A pool
            ps2 = tc.tile_pool(name="ps2", bufs=1, space="PSUM")
            ps2p = ps2.__enter__()
            pA1 = tc.tile_pool(name="pA1", bufs=1, space="PSUM")
            pA1p = pA1.__enter__()
            # pin PSUM address order: s(4K) a0(2K) a1(2K) bc(2K) | ps(4K) tp
            _ = ps2p.tile([128, 2, 512], f32, tag="s", bufs=1)
            _ = ps2p.tile([128, 512], f32, tag="a0", bufs=1)
            _ = ps2p.tile([128, 512], f32, tag="a1", bufs=1)
            _ = ps2p.tile([128, 512], f32, tag="bc", bufs=1)

            emit_A_tile(14, pA1p, pA1p, 1, False, tpbufs=1)
            emit_B_block(0, 0, ps2p, TPB)
            emit_A_tile(15, pA1p, pA1p, 1, False, tpbufs=1)
            emit_B_block(0, 1, ps2p, TPB)
            emit_B_block(0, 2, ps2p, TPB)
            emit_B_block(0, 3, ps2p, TPB)
            for ti in range(TPB):
                gt = TPB + ti
                emit_A_tile(gt, pA1p, pA1p, 1, False, tpbufs=1)
                if ti >= 3 and (ti - 3) % 4 == 0 and (ti - 3) // 4 < 3:
                    emit_B_block(1, (ti - 3) // 4, ps2p, TPB)
            emit_B_block(1, 3, ps2p, TPB)
            pA1.__exit__(None, None, None)

            # ---------------- section 3: out-projection (fills exp drain)
            with tc.tile_pool(name="opool", bufs=3, space="PSUM") as opool:
                for b in range(B):
                    for qb in range(NQB):
                        aT0 = aTs[(b, qb, 0)]
                        aT1 = aTs[(b, qb, 1)]
                        for j in range(4):
                            ti = b * TPB + qb * 4 + j
                            for oc in range(4):
                                ops = opool.tile([128, 512], f32, tag="o")
                                nc.tensor.matmul(
                                    ops, aT0[:, j * 128:(j + 1) * 128],
                                    wo_sb[:, 0, oc * 512:(oc + 1) * 512],
                                    start=True, stop=False)
                                nc.tensor.matmul(
                                    ops, aT1[:, j * 128:(j + 1) * 128],
                                    wo_sb[:, 1, oc * 512:(oc + 1) * 512],
                                    start=False, stop=True)
                                ob = obpool.tile([128, 512], bf16, tag="ob")
                                if oc % 2 == 0:
                                    nc.scalar.copy(ob, ops)
                                else:
                                    nc.vector.tensor_copy(ob, ops)
                                nc.sync.dma_start(
                                    out=out_d[ti * 128:(ti + 1) * 128,
                                              oc * 512:(oc + 1) * 512],
                                    in_=ob)
            ps2.__exit__(None, None, None)

            for p in reversed(sb_pools):
                p.__exit__(None, None, None)
    nc.finalize()
    return nc


# --------------------------------------------------------------------------
# Host-side prep
# --------------------------------------------------------------------------

def _host_prep(hidden_states, qkv_w, o_w, position_ids, S=S_FULL):
    """Returns (shared dict, per-core list of dicts) of numpy arrays."""
    T = B * S
    x = np.ascontiguousarray(hidden_states.reshape(T, H), dtype=np.float32)
    xT = np.ascontiguousarray(x.T).astype(nbf16)

    pos = np.asarray(position_ids).reshape(T).astype(np.float64)
    inv = THETA ** (-np.arange(0, ROT, 2, dtype=np.float64) / ROT)  # [8]
    f = pos[:, None] * inv[None, :]                                 # [T, 8]
    emb = np.concatenate([f, f], axis=1)                            # [T, 16]
    TT = T // 128
    # packed per-partition-linear layout [128, TT, 32]: cos | sin
    cs = np.empty((128, TT, 2 * ROT), np.float32)
    cs[:, :, 0:ROT] = np.cos(emb).reshape(TT, 128, ROT).transpose(1, 0, 2)
    cs[:, :, ROT:2 * ROT] = np.sin(emb).reshape(TT, 128, ROT).transpose(1, 0, 2)
    csd = np.ascontiguousarray(cs.reshape(128, TT * 2 * ROT)).astype(nbf16)

    # mask[p, j]: 1 when q offset j >= k offset p else 0 (applied post-exp)
    p_idx = np.arange(128)[:, None]
    j_idx = np.arange(128)[None, :]
    maskd = np.ascontiguousarray(
        np.where(j_idx >= p_idx, 1.0, 0.0)).astype(nbf16)

    sel33 = np.zeros((33, 128), np.float32)
    sel33[0, 0:64] = 1.0
    sel33[32, 64:128] = 1.0
    sel33d = np.ascontiguousarray(sel33).astype(nbf16)

    shared = {"xT": xT, "csd": csd, "maskd": maskd, "sel33d": sel33d}

    qkv = np.asarray(qkv_w, dtype=np.float32)
    ow = np.asarray(o_w, dtype=np.float32)
    scale = 1.0 / np.sqrt(HD)
    per_core = []
    for c in range(NCORES):
        cols = np.empty((768, H), np.float32)
        for t in range(4):                    # qk dim-tiles
            qk_sel = 0 if t < 2 else 1        # 0 = q, 1 = k
            for u in range(2):
                hl = 2 * (t % 2) + u
                hg = HPC * c + hl
                w = qkv[qk_sel * H + hg * HD: qk_sel * H + (hg + 1) * HD]
                if qk_sel == 0:
                    w = w * scale
                cols[t * 128 + u * 64: t * 128 + u * 64 + 64] = w
        for hl in range(HPC):                 # v dims
            hg = HPC * c + hl
            cols[512 + hl * 64: 512 + (hl + 1) * 64] = \
                qkv[2 * H + hg * HD: 2 * H + (hg + 1) * HD]
        wqkvT = np.ascontiguousarray(cols.T).astype(nbf16)
        woT = np.ascontiguousarray(
            ow[:, LDIM * c: LDIM * (c + 1)].T).astype(nbf16)
        per_core.append({"wqkvT": wqkvT, "woT": woT})
    return shared, per_core


_NC_CACHE = {}


def _get_nc(S=S_FULL):
    if S not in _NC_CACHE:
        _NC_CACHE[S] = build_nc(S)
    return _NC_CACHE[S]


def _run(hidden_states, qkv_w, o_w, position_ids, S=S_FULL, trace=False,
         trace_kwargs=None):
    shared, per_core = _host_prep(hidden_states, qkv_w, o_w, position_ids, S)
    in_maps = [{**shared, **per_core[c]} for c in range(NCORES)]
    nc = _get_nc(S)
    br = run_bass_kernel_spmd(
        nc, in_maps, list(range(NCORES)), trace=trace,
        **(trace_kwargs or {}))
    T = B * S
    out = np.zeros((T, H), np.float32)
    for r in br.results:
        out += r["out"].astype(np.float32)
    return out.reshape(B, S, H), br


def kernel(hidden_states, qkv_w, o_w, position_ids):
    out, _ = _run(hidden_states, qkv_w, o_w, position_ids)
    return out


# revision 17
# speedup vs baseline: 1.1583x; 1.1583x over previous
"""Trainium2 Bass kernel for GPT-NeoX-style attention block (nn_Attention_88141318848873).

Full inputs -> head-parallel tensor-parallel across 8 NeuronCores -> full output.

v2: merged A/B software pipeline.
  - Phase A (QKV proj + partial RoPE + PE-transpose to [dim, tok]) emits per
    128-token tile; phase B blocks (scores -> exp -> PV) are emitted as soon as
    their k/q tiles are transposed, so the exp stream (ScalarE, ~160us total)
    hides under the PE-bound QKV stream instead of pacing a separate phase.
  - PV operand swap: V_aug [k, 64+ones] is the stationary operand, P^T [k, q]
    the moving one -> acc = attn^T [dims+den, q] accumulates over ki with a
    natural start=True bank-open (no zeroing matmuls) and the out-projection
    consumes acc's layout directly (no PE transposes of attn).
  - Normalization in [dims, q] layout: DVE reciprocal of the two denominator
    rows -> bf16 [2, 512] -> K=2 selector matmul (sel2^T @ recb) broadcasts
    recipA to partitions 0..63 / recipB to 64..127 in one 512-cycle matmul ->
    two mixed-partition-base DVE muls produce the bf16 aT pair tile.
  - All out-proj chunks deferred to the tail section (priority-last): the list
    scheduler uses them to fill PE slack while the final blocks' exp drains.
  - PSUM is sectioned (16KB/partition, bank-aligned by construction):
    s1 A-only double-buffered [0:9K); s2 scores 4K + acc 2x2K + bc 2K @0 with
    single-buffered A pool @10K; s3 swaps the A pool for the out-proj pool.
  - PE warmup: ~20 zero matmuls + a dummy exp (ACT table preload) at priority
    0 run during the initial DMA wait, flipping the HAM clock gate to 2.4GHz
    before the first real matmul; first xt tile DMA'd in quarters.
Host: shards/pre-transposes/casts inputs (scale 1/sqrt(hd) folded into Wq,
rope cos/sin tables partition-linear), sums the 8 bf16 partial outputs in f32.
"""
import sys

sys.path.insert(0, "/opt/trn_rl_repo")

import numpy as np
import ml_dtypes

import concourse.bass as bass
import concourse.mybir as mybir
import concourse.tile as tile
from concourse.bacc import Bacc
from concourse.bass_utils import run_bass_kernel_spmd
from concourse.masks import make_identity

B, S_FULL, H = 2, 2048, 2048
NH, HD, ROT = 32, 64, 16
THETA = 10000.0
NCORES = 8
HPC = NH // NCORES            # heads per core = 4
LDIM = HPC * HD               # local attn dims = 256
NEG = -1e30

bf16 = mybir.dt.bfloat16
f32 = mybir.dt.float32
nbf16 = ml_dtypes.bfloat16
Exp = mybir.ActivationFunctionType.Exp


# --------------------------------------------------------------------------
# Bass program (identical on every core; per-core tensors differ)
# --------------------------------------------------------------------------

def build_nc(S=S_FULL):
    assert S % 512 == 0
    T = B * S
    TT = T // 128                 # token tiles total
    TPB = S // 128                # token tiles per batch
    NQB = S // 512                # 512-wide q blocks per batch
    HC = H // 128                 # h (contraction) chunks

    nc = Bacc()
    xT_d = nc.dram_tensor("xT", [H, T], bf16, kind="ExternalInput")
    wqkv_d = nc.dram_tensor("wqkvT", [H, 768], bf16, kind="ExternalInput")
    wo_d = nc.dram_tensor("woT", [LDIM, H], bf16, kind="ExternalInput")
    cs_d = nc.dram_tensor("csd", [128, TT * 2 * ROT], bf16, kind="ExternalInput")
    mask_d = nc.dram_tensor("maskd", [128, 128], bf16, kind="ExternalInput")
    sel_d = nc.dram_tensor("sel33d", [33, 128], bf16, kind="ExternalInput")
    out_d = nc.dram_tensor("out", [T, H], bf16, kind="ExternalOutput")

    with tile.TileContext(nc) as tc:
        with tc.tile_pool(name="const", bufs=1) as cpool:
            wqkv_sb = cpool.tile([128, HC, 768], bf16)
            wqkv_r = wqkv_d.rearrange("(c p) d -> p c d", p=128)
            with tc.high_priority():
                nc.sync.dma_start(out=wqkv_sb[:, 0, :], in_=wqkv_r[:, 0, :])
            for hcd in range(1, HC):
                nc.sync.dma_start(
                    out=wqkv_sb[:, hcd, :], in_=wqkv_r[:, hcd, :])
            wo_sb = cpool.tile([128, 2, H], bf16)   # DMA deferred past startup
            cs_sb = cpool.tile([128, TT, 2 * ROT], bf16)
            nc.sync.dma_start(
                out=cs_sb, in_=cs_d.rearrange("p (t d) -> p t d", d=2 * ROT))
            mask_sb = cpool.tile([128, 128], bf16)   # 0/1 keep-mask, post-exp
            nc.sync.dma_start(out=mask_sb, in_=mask_d[:, :])
            ident = cpool.tile([128, 128], bf16)
            make_identity(nc, ident)
            zeros_sb = cpool.tile([128, 512], bf16)
            nc.vector.memset(zeros_sb, 0.0)
            # selector for the K=33 recip-broadcast matmul: row 0 selects
            # out partitions 0..63 (recipA), row 32 selects 64..127 (recipB)
            sel33_sb = cpool.tile([33, 128], bf16)
            nc.sync.dma_start(out=sel33_sb, in_=sel_d[:, :])

            qkT_sb = cpool.tile([128, 4, T], bf16)       # dims x tok (4 dtiles)
            V_sb = cpool.tile([128, TT, HPC, 66], bf16)  # tok x head x (64+one)
            nc.vector.memset(V_sb[:, :, :, 64:65], 1.0)

            # PE warmup + ACT exp-table preload during the startup DMA wait
            dumm = cpool.tile([1, 1], bf16)
            with tc.tile_pool(name="warm", bufs=1, space="PSUM") as wpool:
                wps = wpool.tile([128, 512], f32, tag="w")
                with tc.high_priority():
                    nc.scalar.activation(out=dumm, in_=zeros_sb[0:1, 0:1],
                                         func=Exp)
                    for _ in range(20):
                        nc.tensor.matmul(wps, zeros_sb[:, 0:128], zeros_sb,
                                         start=True, stop=True)

            sb_pools = [
                tc.tile_pool(name="xt", bufs=4),
                tc.tile_pool(name="qknat", bufs=3),
                tc.tile_pool(name="ropetmp", bufs=4),
                tc.tile_pool(name="ppool", bufs=6),
                tc.tile_pool(name="anpool", bufs=16),
                tc.tile_pool(name="recpool", bufs=4),
                tc.tile_pool(name="obpool", bufs=6),
            ]
            xpool, qpool, rpool, ppool, anpool, recpool, obpool = \
                [p.__enter__() for p in sb_pools]

            xT_r = xT_d.rearrange("(c p) t -> p c t", p=128)

            def emit_A_tile(gt, pspool, tppool, psbufs, copies_on_scalar,
                            tpbufs=4):
                xt = xpool.tile([128, HC, 128], bf16, tag="xt")
                if gt == 0:
                    with tc.high_priority():
                        for qtr in range(4):
                            nc.sync.dma_start(
                                out=xt[:, qtr * 4:(qtr + 1) * 4, :],
                                in_=xT_r[:, qtr * 4:(qtr + 1) * 4,
                                         0:128])
                elif gt < 4:
                    with tc.high_priority():
                        nc.sync.dma_start(
                            out=xt, in_=xT_r[:, :, gt * 128:(gt + 1) * 128])
                else:
                    nc.sync.dma_start(
                        out=xt, in_=xT_r[:, :, gt * 128:(gt + 1) * 128])
                ps = pspool.tile([128, 512], f32, tag="ps", bufs=psbufs)
                for hc in range(HC):
                    nc.tensor.matmul(
                        ps, xt[:, hc, :], wqkv_sb[:, hc, 0:512],
                        start=(hc == 0), stop=(hc == HC - 1))
                qk = qpool.tile([128, 512], bf16, tag="qk")
                nc.vector.tensor_copy(qk, ps)
                psv = pspool.tile([128, 512], f32, tag="ps", bufs=psbufs)
                for hc in range(HC):
                    nc.tensor.matmul(
                        psv[:, 0:256], xt[:, hc, :], wqkv_sb[:, hc, 512:768],
                        start=(hc == 0), stop=(hc == HC - 1))
                nc.vector.tensor_copy(
                    V_sb[:, gt, :, 0:64],
                    psv[:, 0:256].rearrange("p (h d) -> p h d", d=64))
                # partial RoPE on dims 0..15 of each of the 8 (q/k, head) blocks
                rot = qk.rearrange("p (b d) -> p b d", d=64)[:, :, 0:ROT]
                rot_lo = qk.rearrange("p (b d) -> p b d", d=64)[:, :, 0:8]
                rot_hi = qk.rearrange("p (b d) -> p b d", d=64)[:, :, 8:16]
                cos_bc = cs_sb[:, gt, None, 0:ROT].broadcast_to([128, 8, ROT])
                sin_lo = cs_sb[:, gt, None, ROT:ROT + 8].broadcast_to([128, 8, 8])
                sin_hi = cs_sb[:, gt, None, ROT + 8:ROT + 16].broadcast_to([128, 8, 8])
                tmp = rpool.tile([128, 8, ROT], bf16, tag="t0")
                t2l = rpool.tile([128, 8, 8], bf16, tag="t1")
                t2h = rpool.tile([128, 8, 8], bf16, tag="t2")
                nc.vector.tensor_mul(tmp, rot, cos_bc)
                nc.vector.tensor_mul(t2l, rot_hi, sin_lo)
                nc.vector.tensor_mul(t2h, rot_lo, sin_hi)
                nc.vector.tensor_sub(rot_lo, tmp[:, :, 0:8], t2l)
                nc.vector.tensor_add(rot_hi, tmp[:, :, 8:16], t2h)
                # transpose the 4 dim-tiles into qkT
                for dt in range(4):
                    tp = tppool.tile([128, 128], bf16, tag="tp", bufs=tpbufs)
                    nc.tensor.transpose(
                        tp, qk[:, dt * 128:(dt + 1) * 128], ident)
                    if copies_on_scalar and dt % 2 == 1:
                        nc.scalar.copy(
                            qkT_sb[:, dt, gt * 128:(gt + 1) * 128], tp)
                    else:
                        nc.vector.tensor_copy(
                            qkT_sb[:, dt, gt * 128:(gt + 1) * 128], tp)

            aTs = {}

            def emit_B_block(b, qb, spool, TPB):
                for pr in range(2):
                    hA, hB = 2 * pr, 2 * pr + 1
                    accA = spool.tile([128, 512], f32, tag="a0", bufs=1)
                    accB = spool.tile([128, 512], f32, tag="a1", bufs=1)
                    nki = 4 * qb + 4
                    for ki in range(nki):
                        off = max(0, ki * 128 - qb * 512)
                        kcol = b * S + ki * 128
                        qcol = b * S + qb * 512
                        sAB = spool.tile([128, 2, 512], f32, tag="s", bufs=2)
                        with tc.high_priority(offset=150):
                            nc.tensor.matmul(
                                sAB[:, 0, off:512],
                                qkT_sb[0:64, 2 + pr, kcol:kcol + 128],
                                qkT_sb[0:64, pr, qcol + off:qcol + 512],
                                start=True, stop=True,
                                tile_position=(0, 0))
                            nc.tensor.matmul(
                                sAB[:, 1, off:512],
                                qkT_sb[64:128, 2 + pr, kcol:kcol + 128],
                                qkT_sb[64:128, pr, qcol + off:qcol + 512],
                                start=True, stop=True,
                                tile_position=(64, 0))
                        pAB = ppool.tile([128, 2, 512], bf16, tag="p")
                        nc.scalar.activation(
                            out=pAB[:, :, off:512],
                            in_=sAB[:, :, off:512], func=Exp)
                        if ki * 128 >= qb * 512:  # in-block diagonal
                            mask2 = mask_sb[:, None, :].broadcast_to(
                                [128, 2, 128])
                            nc.vector.tensor_mul(
                                pAB[:, :, off:off + 128],
                                pAB[:, :, off:off + 128], mask2)
                        last = (ki == nki - 1)
                        ti = b * TPB + ki
                        nc.tensor.matmul(
                            accA[0:65, off:512],
                            V_sb[:, ti, hA, 0:65], pAB[:, 0, off:512],
                            start=(ki == 0), stop=last)
                        nc.tensor.matmul(
                            accB[0:65, off:512],
                            V_sb[:, ti, hB, 0:65], pAB[:, 1, off:512],
                            start=(ki == 0), stop=last)
                    # normalize -> aT pair tile [128 dims(A|B), 512 q]
                    rec33 = recpool.tile([33, 512], f32, tag="rec")
                    nc.vector.reciprocal(rec33[0:1, :], accA[64:65, :])
                    nc.vector.reciprocal(rec33[32:33, :], accB[64:65, :])
                    recb33 = recpool.tile([33, 512], bf16, tag="recb")
                    nc.vector.memset(recb33, 0.0)
                    nc.vector.tensor_copy(recb33[0:1, :], rec33[0:1, :])
                    nc.vector.tensor_copy(recb33[32:33, :], rec33[32:33, :])
                    # bc rides the score-tile ring (same tag/shape) to save
                    # a PSUM bank; one extra generation per norm
                    bc2 = spool.tile([128, 2, 512], f32, tag="s", bufs=2)
                    bc = bc2[:, 0, :]
                    nc.tensor.matmul(bc, sel33_sb, recb33,
                                     start=True, stop=True)
                    # DVE cannot read two PSUM operands -> stage bc in SBUF
                    bcs = recpool.tile([128, 512], f32, tag="bcs")
                    nc.vector.tensor_copy(bcs, bc)
                    aT = anpool.tile([128, 512], bf16, tag="aT")
                    nc.vector.tensor_mul(aT[0:64, :], accA[0:64, :],
                                         bcs[0:64, :])
                    nc.vector.tensor_mul(aT[64:128, :], accB[0:64, :],
                                         bcs[64:128, :])
                    aTs[(b, qb, pr)] = aT

            # ---------------- section 1: A(b0) tiles 0..13, PSUM double-buf
            pA2 = tc.tile_pool(name="pA2", bufs=1, space="PSUM")
            pA2p = pA2.__enter__()
            for gt in range(0, 14):
                emit_A_tile(gt, pA2p, pA2p, 4, True)
            pA2.__exit__(None, None, None)

            # wo load deferred here so startup DMA bandwidth goes to xT/wqkv
            nc.sync.dma_start(
                out=wo_sb, in_=wo_d.rearrange("(c p) d -> p c d", p=128))

            # ---------------- section 2: B pools + single-buffered A pool
            ps2 = tc.tile_pool(name="ps2", bufs=1, space="PSUM")
            ps2p = ps2.__enter__()
            pA1 = tc.tile_pool(name="pA1", bufs=1, space="PSUM")
            pA1p = pA1.__enter__()
            # pin PSUM address order: s(2x4K) a0(2K) a1(2K) | ps(2K) tp(2K)
            _ = ps2p.tile([128, 2, 512], f32, tag="s", bufs=2)
            _ = ps2p.tile([128, 512], f32, tag="a0", bufs=1)
            _ = ps2p.tile([128, 512], f32, tag="a1", bufs=1)

            emit_A_tile(14, pA1p, pA1p, 1, False, tpbufs=1)
            emit_B_block(0, 0, ps2p, TPB)
            emit_A_tile(15, pA1p, pA1p, 1, False, tpbufs=1)
            emit_B_block(0, 1, ps2p, TPB)
            emit_B_block(0, 2, ps2p, TPB)
            emit_B_block(0, 3, ps2p, TPB)
            for ti in range(TPB):
                gt = TPB + ti
                emit_A_tile(gt, pA1p, pA1p, 1, False, tpbufs=1)
                if ti >= 3 and (ti - 3) % 4 == 0 and (ti - 3) // 4 < 3:
                    emit_B_block(1, (ti - 3) // 4, ps2p, TPB)
            emit_B_block(1, 3, ps2p, TPB)
            pA1.__exit__(None, None, None)

            # ---------------- section 3: out-projection (fills exp drain)
            with tc.tile_pool(name="opool", bufs=2, space="PSUM") as opool:
                for b in range(B):
                    for qb in range(NQB):
                        aT0 = aTs[(b, qb, 0)]
                        aT1 = aTs[(b, qb, 1)]
                        for j in range(4):
                            ti = b * TPB + qb * 4 + j
                            for oc in range(4):
                                ops = opool.tile([128, 512], f32, tag="o")
                                nc.tensor.matmul(
                                    ops, aT0[:, j * 128:(j + 1) * 128],
                                    wo_sb[:, 0, oc * 512:(oc + 1) * 512],
                                    start=True, stop=False)
                                nc.tensor.matmul(
                                    ops, aT1[:, j * 128:(j + 1) * 128],
                                    wo_sb[:, 1, oc * 512:(oc + 1) * 512],
                                    start=False, stop=True)
                                ob = obpool.tile([128, 512], bf16, tag="ob")
                                if oc % 2 == 0:
                                    nc.scalar.copy(ob, ops)
                                else:
                                    nc.vector.tensor_copy(ob, ops)
                                nc.sync.dma_start(
                                    out=out_d[ti * 128:(ti + 1) * 128,
                                              oc * 512:(oc + 1) * 512],
                                    in_=ob)
            ps2.__exit__(None, None, None)

            for p in reversed(sb_pools):
                p.__exit__(None, None, None)
    nc.finalize()
    return nc


# --------------------------------------------------------------------------
# Host-side prep
# --------------------------------------------------------------------------

def _host_prep(hidden_states, qkv_w, o_w, position_ids, S=S_FULL):
    """Returns (shared dict, per-core list of dicts) of numpy arrays."""
    T = B * S
    x = np.ascontiguousarray(hidden_states.reshape(T, H), dtype=np.float32)
    xT = np.ascontiguousarray(x.T).astype(nbf16)

    pos = np.asarray(position_ids).reshape(T).astype(np.float64)
    inv = THETA ** (-np.arange(0, ROT, 2, dtype=np.float64) / ROT)  # [8]
    f = pos[:, None] * inv[None, :]                                 # [T, 8]
    emb = np.concatenate([f, f], axis=1)                            # [T, 16]
    TT = T // 128
    # packed per-partition-linear layout [128, TT, 32]: cos | sin
    cs = np.empty((128, TT, 2 * ROT), np.float32)
    cs[:, :, 0:ROT] = np.cos(emb).reshape(TT, 128, ROT).transpose(1, 0, 2)
    cs[:, :, ROT:2 * ROT] = np.sin(emb).reshape(TT, 128, ROT).transpose(1, 0, 2)
    csd = np.ascontiguousarray(cs.reshape(128, TT * 2 * ROT)).astype(nbf16)

    # mask[p, j]: 1 when q offset j >= k offset p else 0 (applied post-exp)
    p_idx = np.arange(128)[:, None]
    j_idx = np.arange(128)[None, :]
    maskd = np.ascontiguousarray(
        np.where(j_idx >= p_idx, 1.0, 0.0)).astype(nbf16)

    sel33 = np.zeros((33, 128), np.float32)
    sel33[0, 0:64] = 1.0
    sel33[32, 64:128] = 1.0
    sel33d = np.ascontiguousarray(sel33).astype(nbf16)

    shared = {"xT": xT, "csd": csd, "maskd": maskd, "sel33d": sel33d}

    qkv = np.asarray(qkv_w, dtype=np.float32)
    ow = np.asarray(o_w, dtype=np.float32)
    scale = 1.0 / np.sqrt(HD)
    per_core = []
    for c in range(NCORES):
        cols = np.empty((768, H), np.float32)
        for t in range(4):                    # qk dim-tiles
            qk_sel = 0 if t < 2 else 1        # 0 = q, 1 = k
            for u in range(2):
                hl = 2 * (t % 2) + u
                hg = HPC * c + hl
                w = qkv[qk_sel * H + hg * HD: qk_sel * H + (hg + 1) * HD]
                if qk_sel == 0:
                    w = w * scale
                cols[t * 128 + u * 64: t * 128 + u * 64 + 64] = w
        for hl in range(HPC):                 # v dims
            hg = HPC * c + hl
            cols[512 + hl * 64: 512 + (hl + 1) * 64] = \
                qkv[2 * H + hg * HD: 2 * H + (hg + 1) * HD]
        wqkvT = np.ascontiguousarray(cols.T).astype(nbf16)
        woT = np.ascontiguousarray(
            ow[:, LDIM * c: LDIM * (c + 1)].T).astype(nbf16)
        per_core.append({"wqkvT": wqkvT, "woT": woT})
    return shared, per_core


_NC_CACHE = {}


def _get_nc(S=S_FULL):
    if S not in _NC_CACHE:
        _NC_CACHE[S] = build_nc(S)
    return _NC_CACHE[S]


def _run(hidden_states, qkv_w, o_w, position_ids, S=S_FULL, trace=False,
         trace_kwargs=None):
    shared, per_core = _host_prep(hidden_states, qkv_w, o_w, position_ids, S)
    in_maps = [{**shared, **per_core[c]} for c in range(NCORES)]
    nc = _get_nc(S)
    br = run_bass_kernel_spmd(
        nc, in_maps, list(range(NCORES)), trace=trace,
        **(trace_kwargs or {}))
    T = B * S
    out = np.zeros((T, H), np.float32)
    for r in br.results:
        out += r["out"].astype(np.float32)
    return out.reshape(B, S, H), br


def kernel(hidden_states, qkv_w, o_w, position_ids):
    out, _ = _run(hidden_states, qkv_w, o_w, position_ids)
    return out
